# revision 9
# baseline (speedup 1.0000x reference)
"""Trainium2 kernel for nn_CompositeOneGRU (gnn_message_passing).

Math notes (derived from the reference):
  - Only row 0 of each sample's GCN state feeds the output heads
    (x1 = relu(new_mem)[0]), and `proposed` depends only on the current
    sample's features, so the per-sample message passing collapses to a
    [6,32] scatter/degree matrix G_b per sample with
        proposed_b[0,:] = concat_r(G_b[r] @ x_b) @ W_flat,
    W_flat = vstack(conv_W[0..4], W0)  (all index work is host preprocessing).
  - update_gate == 1 makes the scan carry vacuous; the general case is
    handled exactly on the host (it is linear pre-relu).
  - Device work (the heavy part): X1^T = relu(W_flat^T @ Y^T), then the two
    vocab heads sharded 8-way over the vocabulary (6250 glob + 2500 sense
    columns per core).  Raw logits stay in SBUF (f32); ScalarE computes
    exp with accumulated row-sums, one AllReduce per head combines the
    softmax denominators, and VectorE writes  out = logit - lse  as f16.
"""

import sys

sys.path.insert(0, "/opt/trn_rl_repo")

import numpy as np

import concourse.bacc as bacc
import concourse.bass as bass
import concourse.tile as tile
import concourse.mybir as mybir
from concourse.bass_utils import run_bass_kernel_spmd

B, N, D, R, E = 512, 32, 300, 5, 256
VG, VS = 50000, 20000
NCORES = 8
VG_SH, VS_SH = VG // NCORES, VS // NCORES      # 6250 / 2500 per core
W_SH = VG_SH + VS_SH                            # 8750 logit cols per core
NTILE = 350                                     # uniform n-tile (25 x 350)
NNT = W_SH // NTILE                             # 25
NGRP, GSZ = 5, 5                                # whead DMA groups: 5 x 5 tiles
K = 6 * D                                       # 1800
KC = 120                                        # stage-A k-chunk
NMT = B // 128                                  # 4 sample tiles
# head-respecting column ranges for exp / subtract / output staging
RANGES_G = [(0, 1750), (1750, 3500), (3500, 5250), (5250, 6250)]
RANGES_S = [(6250, 8000), (8000, 8750)]

f32 = mybir.dt.float32
f16 = mybir.dt.float16
AF = mybir.ActivationFunctionType

_CACHE = {}


def _build_device():
    nc = bacc.Bacc("TRN2", target_bir_lowering=False, debug=False,
                   num_devices=NCORES)

    yT = nc.dram_tensor("yT", [KC, 15, B], f16, kind="ExternalInput")
    wflat = nc.dram_tensor("wflat", [KC, 15, D], f16, kind="ExternalInput")
    wh_r0 = nc.dram_tensor("wh_r0", [128, W_SH], f16, kind="ExternalInput")
    wh_r1 = nc.dram_tensor("wh_r1", [128, W_SH], f16, kind="ExternalInput")
    wh_r2 = nc.dram_tensor("wh_r2", [45, W_SH], f16, kind="ExternalInput")
    ones_d = nc.dram_tensor("ones", [1, B], f16, kind="ExternalInput")
    out = nc.dram_tensor("out", [B, W_SH], f16, kind="ExternalOutput")

    DT = [(0, 128), (128, 256), (256, 300)]
    NSPL = 2048          # first-chunk split so head matmuls can start early

    with tile.TileContext(nc) as tc:
        with (
            tc.tile_pool(name="sba", bufs=1) as sba,
            tc.tile_pool(name="sbw", bufs=1) as sbw,
            tc.tile_pool(name="sbl", bufs=1) as sbl,
            tc.tile_pool(name="sbx", bufs=1) as sbx,
            tc.tile_pool(name="sbo", bufs=4) as sbo,
            tc.tile_pool(name="psa", bufs=2, space="PSUM") as psa,
            tc.tile_pool(name="psh", bufs=6, space="PSUM") as psh,
            tc.tile_pool(name="dram", bufs=1, space="DRAM") as dram,
        ):
            # ------------- stage A: X1^T = relu(W_flat^T @ Y^T) -------------
            ytc = sba.tile([KC, 15, B], f16, name="ytc")
            wfc = sba.tile([KC, 15, D], f16, name="wfc")
            nc.sync.dma_start(out=ytc[:], in_=yT.ap())
            nc.sync.dma_start(out=wfc[:], in_=wflat.ap())
            x1c = sbx.tile([128, 3 * B], f16, name="x1c")
            # compute ops need quadrant-aligned partition starts; DMA doesn't
            nc.sync.dma_start(out=x1c[44:45, 2 * B:3 * B], in_=ones_d[:])

            # resident head weights, split loads so col 0 arrives early
            whs = [
                sbw.tile([128, W_SH], f16, name="whs0"),
                sbw.tile([128, W_SH], f16, name="whs1"),
                sbw.tile([45, W_SH], f16, name="whs2"),
            ]
            for t, srcArr in zip(whs, [wh_r0, wh_r1, wh_r2]):
                nc.sync.dma_start(out=t[:, 0:NSPL], in_=srcArr[:, 0:NSPL])
            for t, srcArr in zip(whs, [wh_r0, wh_r1, wh_r2]):
                nc.sync.dma_start(out=t[:, NSPL:], in_=srcArr[:, NSPL:])

            x1 = [x1c[:, 0:B], x1c[:, B:2 * B], x1c[0:45, 2 * B:3 * B]]
            for dt, (d0, d1) in enumerate(DT):
                dk = d1 - d0
                pp = psa.tile([dk, B], f32, tag="pp", name="pp")
                for t in range(15):
                    nc.tensor.matmul(
                        pp[:],
                        lhsT=wfc[:, t, d0:d1],
                        rhs=ytc[:, t, :],
                        start=(t == 0), stop=(t == 14))
                nc.scalar.activation(x1[dt][0:dk, :], pp[:], AF.Relu)

            # ------------- stage B: head matmuls, copies, exp sums -----------
            logits = sbl.tile([128, NMT * W_SH], f16, name="logits")
            spart = sba.tile([128, NMT * 8], f32, name="spart")
            spack = sba.tile([128, 8], f32, name="spack")     # [glob x4 | sense x4]
            lse = sba.tile([128, 8], f32, name="lse")

            cc_in_g = dram.tile([128, 4], f32, name="cc_in_g")
            cc_out_g = dram.tile([128, 4], f32, addr_space="Shared", name="cc_out_g")
            cc_in_s = dram.tile([128, 4], f32, name="cc_in_s")
            cc_out_s = dram.tile([128, 4], f32, addr_space="Shared", name="cc_out_s")

            def emit_exp(mt, ri, c0, c1):
                scratch = sba.tile([128, 1800], f16, tag="exps", name="exps",
                                   bufs=2)
                nc.scalar.activation(
                    scratch[:, 0:c1 - c0],
                    logits[:, mt * W_SH + c0:mt * W_SH + c1], AF.Exp,
                    accum_out=spart[:, mt * 8 + ri:mt * 8 + ri + 1])

            def emit_tail(head, ranges, cc_in, cc_out, lcol):
                # pack sums, AllReduce, lse = Ln(S); then subtract + store
                nr = len(ranges)
                r0 = 0 if head == 0 else len(RANGES_G)
                for mt in range(NMT):
                    nc.vector.reduce_sum(
                        spack[:, lcol + mt:lcol + mt + 1],
                        spart[:, mt * 8 + r0:mt * 8 + r0 + nr],
                        axis=mybir.AxisListType.X)
                nc.sync.dma_start(out=cc_in[:], in_=spack[:, lcol:lcol + 4])
                nc.gpsimd.collective_compute(
                    "AllReduce", mybir.AluOpType.add,
                    replica_groups=[list(range(NCORES))],
                    ins=[cc_in.opt()], outs=[cc_out.opt()])
                sg = sba.tile([128, 4], f32, tag="sg", name="sg", bufs=1)
                nc.sync.dma_start(out=sg[:], in_=cc_out[:])
                nc.scalar.activation(lse[:, lcol:lcol + 4], sg[:], AF.Ln)
                for mt in range(NMT):
                    for (c0, c1) in ranges:
                        stg = sbo.tile([128, 1800], f16, tag="stg", name="stg")
                        nc.vector.tensor_scalar_sub(
                            stg[:, 0:c1 - c0],
                            logits[:, mt * W_SH + c0:mt * W_SH + c1],
                            lse[:, lcol + mt:lcol + mt + 1])
                        nc.gpsimd.dma_start(
                            out=out[128 * mt:128 * (mt + 1), c0:c1],
                            in_=stg[:, 0:c1 - c0])

            # 18 n-tiles of 512 (last is 46); exp ranges fire when covered
            NTL = [512] * 17 + [46]
            ncast = 0
            for nt in range(18):
                n0, nsz = 512 * nt, NTL[nt]
                for mt in range(NMT):
                    msl = slice(128 * mt, 128 * (mt + 1))
                    pt = psh.tile([128, 512], f32, tag="pt", name="pt")
                    for i in range(3):
                        nc.tensor.matmul(
                            pt[0:128, 0:nsz], lhsT=x1[i][:, msl],
                            rhs=whs[i][:, n0:n0 + nsz],
                            start=(i == 0), stop=(i == 2))
                    dst = logits[:, mt * W_SH + n0:mt * W_SH + n0 + nsz]
                    if ncast % 3 == 2:
                        nc.scalar.activation(dst, pt[0:128, 0:nsz], AF.Copy)
                    else:
                        nc.vector.tensor_copy(dst, pt[0:128, 0:nsz])
                    ncast += 1
                done = n0 + nsz
                prev = n0
                for ri, (c0, c1) in enumerate(RANGES_G):
                    if prev < c1 <= done:
                        for mt in range(NMT):
                            emit_exp(mt, ri, c0, c1)
                for ri, (c0, c1) in enumerate(RANGES_S):
                    if prev < c1 <= done:
                        for mt in range(NMT):
                            emit_exp(mt, len(RANGES_G) + ri, c0, c1)
                if prev < 6250 <= done:
                    emit_tail(0, RANGES_G, cc_in_g, cc_out_g, 0)
            emit_tail(1, RANGES_S, cc_in_s, cc_out_s, 4)

    nc.compile()
    return nc


def _host_prep(x, edge_index, edge_type, conv_W, W0, update_gate,
               glob_W, glob_b, sense_W, sense_b, memory0):
    x = np.asarray(x, np.float32)
    ei = np.asarray(edge_index)
    et = np.asarray(edge_type).astype(np.int64)
    src, dst = ei[:, 0, :].astype(np.int64), ei[:, 1, :].astype(np.int64)

    bb = np.broadcast_to(np.arange(B)[:, None], (B, E))
    deg = np.ones((B, N, R), np.float32)
    np.add.at(deg, (bb, dst, et), 1.0)
    dinv = 1.0 / np.sqrt(deg)
    coeff = dinv[bb, src, et] * dinv[bb, dst, et]

    G = np.zeros((B, 6, N), np.float32)
    bm, em = np.nonzero(dst == 0)
    np.add.at(G, (bm, et[bm, em], src[bm, em]), coeff[bm, em])
    G[:, :R, 0] += dinv[:, 0, :] ** 2
    G[:, 5, 0] = 1.0

    Yf = np.einsum("bgn,bnd->bgd", G, x).reshape(B, K)
    Wflat = np.concatenate(
        [np.asarray(conv_W, np.float32).reshape(R * D, D),
         np.asarray(W0, np.float32)], axis=0)

    g = float(np.asarray(update_gate).reshape(-1)[0])
    mem0 = np.asarray(memory0, np.float32)
    if g == 1.0 and not np.any(mem0):
        yT_host = np.ascontiguousarray(Yf.T)
        wf_host = Wflat
    else:
        # exact host fallback for the general carry (linear pre-relu)
        P = Yf @ Wflat
        X1 = np.empty((B, D), np.float32)
        carry = mem0[0].copy()
        for b in range(B):
            carry = g * P[b] + (1.0 - g) * carry
            X1[b] = np.maximum(carry, 0.0)
        yT_host = np.zeros((K, B), np.float32)
        yT_host[:D] = X1.T
        wf_host = np.zeros((K, D), np.float32)
        wf_host[:D] = np.eye(D, dtype=np.float32)

    WcatT = np.concatenate(
        [np.asarray(glob_W, np.float32), np.asarray(sense_W, np.float32)], 0).T
    bcat = np.concatenate(
        [np.asarray(glob_b, np.float32), np.asarray(sense_b, np.float32)], 0)

    wh_r0, wh_r1, wh_r2 = [], [], []
    for c in range(NCORES):
        blk = np.empty((D + 1, W_SH), np.float32)
        g0 = VG_SH * c
        blk[:D, :VG_SH] = WcatT[:, g0:g0 + VG_SH]
        blk[D, :VG_SH] = bcat[g0:g0 + VG_SH]
        s0 = VG + VS_SH * c
        blk[:D, VG_SH:] = WcatT[:, s0:s0 + VS_SH]
        blk[D, VG_SH:] = bcat[s0:s0 + VS_SH]
        blk16 = blk.astype(np.float16)
        wh_r0.append(np.ascontiguousarray(blk16[0:128]))
        wh_r1.append(np.ascontiguousarray(blk16[128:256]))
        wh_r2.append(np.ascontiguousarray(blk16[256:301]))

    # stage-A operands in on-chip chunk layout [KC, 15, cols]
    yt_dev = np.ascontiguousarray(
        yT_host.reshape(15, KC, B).transpose(1, 0, 2)).astype(np.float16)
    wf_dev = np.ascontiguousarray(
        wf_host.reshape(15, KC, D).transpose(1, 0, 2)).astype(np.float16)
    return yt_dev, wf_dev, wh_r0, wh_r1, wh_r2


def kernel(**inputs):
    if "nc" not in _CACHE:
        _CACHE["nc"] = _build_device()
    nc = _CACHE["nc"]

    yT_np, wf_np, wh_r0, wh_r1, wh_r2 = _host_prep(**inputs)
    ones_np = np.ones((1, B), np.float16)
    in_maps = [{"yT": yT_np, "wflat": wf_np, "wh_r0": wh_r0[c],
                "wh_r1": wh_r1[c], "wh_r2": wh_r2[c], "ones": ones_np}
               for c in range(NCORES)]

    import os
    trace = bool(int(os.environ.get("KERNEL_TRACE", "0")))
    res = run_bass_kernel_spmd(nc, in_maps, core_ids=list(range(NCORES)),
                               trace=trace)
    _CACHE["last_result"] = res

    outs = [res.results[c]["out"].astype(np.float32) for c in range(NCORES)]
    glob = np.concatenate([o[:, :VG_SH] for o in outs], axis=1)
    sense = np.concatenate([o[:, VG_SH:] for o in outs], axis=1)
    return glob, sense


# revision 11
# speedup vs baseline: 1.0959x; 1.0959x over previous
"""Trainium2 kernel for nn_CompositeOneGRU (gnn_message_passing).

Math notes (derived from the reference):
  - Only row 0 of each sample's GCN state feeds the output heads
    (x1 = relu(new_mem)[0]), and `proposed` depends only on the current
    sample's features, so the per-sample message passing collapses to a
    [6,32] scatter/degree matrix G_b per sample with
        proposed_b[0,:] = concat_r(G_b[r] @ x_b) @ W_flat,
    W_flat = vstack(conv_W[0..4], W0)  (all index work is host preprocessing).
  - update_gate == 1 makes the scan carry vacuous; the general case is
    handled exactly on the host (it is linear pre-relu).
  - Device work (the heavy part): X1^T = relu(W_flat^T @ Y^T), then the two
    vocab heads sharded 8-way over the vocabulary (6250 glob + 2500 sense
    columns per core).  Raw logits stay in SBUF (f32); ScalarE computes
    exp with accumulated row-sums, one AllReduce per head combines the
    softmax denominators, and VectorE writes  out = logit - lse  as f16.
"""

import sys

sys.path.insert(0, "/opt/trn_rl_repo")

import numpy as np

import concourse.bacc as bacc
import concourse.bass as bass
import concourse.tile as tile
import concourse.mybir as mybir
from concourse.bass_utils import run_bass_kernel_spmd

B, N, D, R, E = 512, 32, 300, 5, 256
VG, VS = 50000, 20000
NCORES = 8
VG_SH, VS_SH = VG // NCORES, VS // NCORES      # 6250 / 2500 per core
W_SH = VG_SH + VS_SH                            # 8750 logit cols per core
NTILE = 350                                     # uniform n-tile (25 x 350)
NNT = W_SH // NTILE                             # 25
NGRP, GSZ = 5, 5                                # whead DMA groups: 5 x 5 tiles
K = 6 * D                                       # 1800
KC = 120                                        # stage-A k-chunk
NMT = B // 128                                  # 4 sample tiles
# head-respecting column ranges for exp / subtract / output staging
RANGES_G = [(0, 1750), (1750, 3500), (3500, 5250), (5250, 6250)]
RANGES_S = [(6250, 8000), (8000, 8750)]

f32 = mybir.dt.float32
f16 = mybir.dt.float16
AF = mybir.ActivationFunctionType

_CACHE = {}


def _build_device():
    nc = bacc.Bacc("TRN2", target_bir_lowering=False, debug=False,
                   num_devices=NCORES)

    yT = nc.dram_tensor("yT", [KC, 15, B], f16, kind="ExternalInput")
    wflat = nc.dram_tensor("wflat", [KC, 15, D], f16, kind="ExternalInput")
    wh_r0 = nc.dram_tensor("wh_r0", [128, W_SH], f16, kind="ExternalInput")
    wh_r1 = nc.dram_tensor("wh_r1", [128, W_SH], f16, kind="ExternalInput")
    wh_r2 = nc.dram_tensor("wh_r2", [45, W_SH], f16, kind="ExternalInput")
    ones_d = nc.dram_tensor("ones", [1, B], f16, kind="ExternalInput")
    out = nc.dram_tensor("out", [B, W_SH], f16, kind="ExternalOutput")

    DT = [(0, 128), (128, 256), (256, 300)]
    NSPL = 2048          # first-chunk split so head matmuls can start early

    with tile.TileContext(nc) as tc:
        with (
            tc.tile_pool(name="sba", bufs=1) as sba,
            tc.tile_pool(name="sbw", bufs=1) as sbw,
            tc.tile_pool(name="sbl", bufs=1) as sbl,
            tc.tile_pool(name="sbx", bufs=1) as sbx,
            tc.tile_pool(name="sbo", bufs=4) as sbo,
            tc.tile_pool(name="psa", bufs=2, space="PSUM") as psa,
            tc.tile_pool(name="psh", bufs=6, space="PSUM") as psh,
            tc.tile_pool(name="dram", bufs=1, space="DRAM") as dram,
        ):
            # warm-up AllReduce: absorbs inter-core launch skew while the
            # input DMAs and stage A run, so the real AR at the tail only
            # pays the mesh-algorithm latency.
            warm = sba.tile([128, 1], f32, name="warm")
            nc.vector.memset(warm[:], 1.0)
            cc_warm_i = dram.tile([128, 1], f32, name="cc_warm_i")
            cc_warm_o = dram.tile([128, 1], f32, addr_space="Shared",
                                  name="cc_warm_o")
            nc.sync.dma_start(out=cc_warm_i[:], in_=warm[:])
            nc.gpsimd.collective_compute(
                "AllReduce", mybir.AluOpType.add,
                replica_groups=[list(range(NCORES))],
                ins=[cc_warm_i.opt()], outs=[cc_warm_o.opt()])

            # ------------- stage A: X1^T = relu(W_flat^T @ Y^T) -------------
            # chunked loads: matmuls start after the first ~0.8MB
            ytc = sba.tile([KC, 15, B], f16, name="ytc")
            wfc = sba.tile([KC, 15, D], f16, name="wfc")
            GRPS = [(0, 4), (4, 8), (8, 12), (12, 15)]
            for (g0, g1) in GRPS:
                nc.sync.dma_start(out=ytc[:, g0:g1, :], in_=yT[:, g0:g1, :])
                nc.sync.dma_start(out=wfc[:, g0:g1, :], in_=wflat[:, g0:g1, :])
            x1c = sbx.tile([128, 3 * B], f16, name="x1c")
            nc.vector.memset(x1c[:], 0.0)
            # compute ops need quadrant-aligned partition starts; DMA doesn't
            nc.sync.dma_start(out=x1c[44:45, 2 * B:3 * B], in_=ones_d[:])

            # resident head weights; whs2 zero-padded to K=128 so all head
            # matmuls are uniform [128,*] (K=45 ran measurably slower)
            whs = [
                sbw.tile([128, W_SH], f16, name="whs0"),
                sbw.tile([128, W_SH], f16, name="whs1"),
                sbw.tile([128, W_SH], f16, name="whs2"),
            ]
            nc.gpsimd.memset(whs[2][:], 0.0)
            for t, srcArr, p in zip(whs, [wh_r0, wh_r1, wh_r2], [128, 128, 45]):
                nc.sync.dma_start(out=t[0:p, 0:NSPL], in_=srcArr[:, 0:NSPL])
            for t, srcArr, p in zip(whs, [wh_r0, wh_r1, wh_r2], [128, 128, 45]):
                nc.sync.dma_start(out=t[0:p, NSPL:], in_=srcArr[:, NSPL:])

            x1 = [x1c[:, 0:B], x1c[:, B:2 * B], x1c[0:45, 2 * B:3 * B]]
            for dt, (d0, d1) in enumerate(DT):
                dk = d1 - d0
                pp = psa.tile([dk, B], f32, tag="pp", name="pp")
                for gi, (g0, g1) in enumerate(GRPS):
                    for t in range(g0, g1):
                        nc.tensor.matmul(
                            pp[:],
                            lhsT=wfc[:, t, d0:d1],
                            rhs=ytc[:, t, :],
                            start=(t == 0), stop=(t == 14))
                nc.scalar.activation(x1[dt][0:dk, :], pp[:], AF.Relu)

            # ------------- stage B: head matmuls, copies, exp sums -----------
            logits = sbl.tile([128, NMT * W_SH], f16, name="logits")
            spart = sba.tile([128, NMT * 8], f32, name="spart")
            spack = sba.tile([128, 8], f32, name="spack")     # [glob x4 | sense x4]
            lse = sba.tile([128, 8], f32, name="lse")

            cc_in_g = dram.tile([128, 8], f32, name="cc_in_g")
            cc_out_g = dram.tile([128, 8], f32, addr_space="Shared", name="cc_out_g")

            def emit_exp(mt, ri, c0, c1):
                scratch = sba.tile([128, 1800], f16, tag="exps", name="exps",
                                   bufs=2)
                nc.scalar.activation(
                    scratch[:, 0:c1 - c0],
                    logits[:, mt * W_SH + c0:mt * W_SH + c1], AF.Exp,
                    accum_out=spart[:, mt * 8 + ri:mt * 8 + ri + 1])

            def emit_tail():
                # pack sums (glob cols 0-3 = mt, sense 4-7), one AllReduce,
                # lse = Ln(S), then subtract + store everything
                for mt in range(NMT):
                    nc.vector.reduce_sum(
                        spack[:, mt:mt + 1],
                        spart[:, mt * 8:mt * 8 + len(RANGES_G)],
                        axis=mybir.AxisListType.X)
                    nc.vector.reduce_sum(
                        spack[:, 4 + mt:5 + mt],
                        spart[:, mt * 8 + len(RANGES_G):mt * 8 + 8],
                        axis=mybir.AxisListType.X)
                nc.sync.dma_start(out=cc_in_g[:], in_=spack[:])
                nc.gpsimd.collective_compute(
                    "AllReduce", mybir.AluOpType.add,
                    replica_groups=[list(range(NCORES))],
                    ins=[cc_in_g.opt()], outs=[cc_out_g.opt()])
                sg = sba.tile([128, 8], f32, tag="sg", name="sg", bufs=1)
                nc.sync.dma_start(out=sg[:], in_=cc_out_g[:])
                nc.scalar.activation(lse[:], sg[:], AF.Ln)
                for mt in range(NMT):
                    for hi, ranges in enumerate([RANGES_G, RANGES_S]):
                        for (c0, c1) in ranges:
                            stg = sbo.tile([128, 1800], f16, tag="stg",
                                           name="stg")
                            nc.vector.tensor_scalar_sub(
                                stg[:, 0:c1 - c0],
                                logits[:, mt * W_SH + c0:mt * W_SH + c1],
                                lse[:, 4 * hi + mt:4 * hi + mt + 1])
                            nc.gpsimd.dma_start(
                                out=out[128 * mt:128 * (mt + 1), c0:c1],
                                in_=stg[:, 0:c1 - c0])

            # 18 n-tiles of 512 (last is 46); exp ranges fire when covered
            NTL = [512] * 17 + [46]
            ncast = 0
            for nt in range(18):
                n0, nsz = 512 * nt, NTL[nt]
                for mt in range(NMT):
                    msl = slice(128 * mt, 128 * (mt + 1))
                    pt = psh.tile([128, 512], f32, tag="pt", name="pt")
                    for i in range(3):
                        nc.tensor.matmul(
                            pt[0:128, 0:nsz],
                            lhsT=x1c[:, i * B + 128 * mt:i * B + 128 * (mt + 1)],
                            rhs=whs[i][:, n0:n0 + nsz],
                            start=(i == 0), stop=(i == 2))
                    dst = logits[:, mt * W_SH + n0:mt * W_SH + n0 + nsz]
                    if ncast % 3 == 2:
                        nc.scalar.activation(dst, pt[0:128, 0:nsz], AF.Copy)
                    else:
                        nc.vector.tensor_copy(dst, pt[0:128, 0:nsz])
                    ncast += 1
                done = n0 + nsz
                prev = n0
                for ri, (c0, c1) in enumerate(RANGES_G):
                    if prev < c1 <= done:
                        for mt in range(NMT):
                            emit_exp(mt, ri, c0, c1)
                for ri, (c0, c1) in enumerate(RANGES_S):
                    if prev < c1 <= done:
                        for mt in range(NMT):
                            emit_exp(mt, len(RANGES_G) + ri, c0, c1)
            emit_tail()

    nc.compile()
    return nc


def _host_prep(x, edge_index, edge_type, conv_W, W0, update_gate,
               glob_W, glob_b, sense_W, sense_b, memory0):
    x = np.asarray(x, np.float32)
    ei = np.asarray(edge_index)
    et = np.asarray(edge_type).astype(np.int64)
    src, dst = ei[:, 0, :].astype(np.int64), ei[:, 1, :].astype(np.int64)

    bb = np.broadcast_to(np.arange(B)[:, None], (B, E))
    deg = np.ones((B, N, R), np.float32)
    np.add.at(deg, (bb, dst, et), 1.0)
    dinv = 1.0 / np.sqrt(deg)
    coeff = dinv[bb, src, et] * dinv[bb, dst, et]

    G = np.zeros((B, 6, N), np.float32)
    bm, em = np.nonzero(dst == 0)
    np.add.at(G, (bm, et[bm, em], src[bm, em]), coeff[bm, em])
    G[:, :R, 0] += dinv[:, 0, :] ** 2
    G[:, 5, 0] = 1.0

    Yf = np.einsum("bgn,bnd->bgd", G, x).reshape(B, K)
    Wflat = np.concatenate(
        [np.asarray(conv_W, np.float32).reshape(R * D, D),
         np.asarray(W0, np.float32)], axis=0)

    g = float(np.asarray(update_gate).reshape(-1)[0])
    mem0 = np.asarray(memory0, np.float32)
    if g == 1.0 and not np.any(mem0):
        yT_host = np.ascontiguousarray(Yf.T)
        wf_host = Wflat
    else:
        # exact host fallback for the general carry (linear pre-relu)
        P = Yf @ Wflat
        X1 = np.empty((B, D), np.float32)
        carry = mem0[0].copy()
        for b in range(B):
            carry = g * P[b] + (1.0 - g) * carry
            X1[b] = np.maximum(carry, 0.0)
        yT_host = np.zeros((K, B), np.float32)
        yT_host[:D] = X1.T
        wf_host = np.zeros((K, D), np.float32)
        wf_host[:D] = np.eye(D, dtype=np.float32)

    WcatT = np.concatenate(
        [np.asarray(glob_W, np.float32), np.asarray(sense_W, np.float32)], 0).T
    bcat = np.concatenate(
        [np.asarray(glob_b, np.float32), np.asarray(sense_b, np.float32)], 0)

    wh_r0, wh_r1, wh_r2 = [], [], []
    for c in range(NCORES):
        blk = np.empty((D + 1, W_SH), np.float32)
        g0 = VG_SH * c
        blk[:D, :VG_SH] = WcatT[:, g0:g0 + VG_SH]
        blk[D, :VG_SH] = bcat[g0:g0 + VG_SH]
        s0 = VG + VS_SH * c
        blk[:D, VG_SH:] = WcatT[:, s0:s0 + VS_SH]
        blk[D, VG_SH:] = bcat[s0:s0 + VS_SH]
        blk16 = blk.astype(np.float16)
        wh_r0.append(np.ascontiguousarray(blk16[0:128]))
        wh_r1.append(np.ascontiguousarray(blk16[128:256]))
        wh_r2.append(np.ascontiguousarray(blk16[256:301]))

    # stage-A operands in on-chip chunk layout [KC, 15, cols]
    yt_dev = np.ascontiguousarray(
        yT_host.reshape(15, KC, B).transpose(1, 0, 2)).astype(np.float16)
    wf_dev = np.ascontiguousarray(
        wf_host.reshape(15, KC, D).transpose(1, 0, 2)).astype(np.float16)
    return yt_dev, wf_dev, wh_r0, wh_r1, wh_r2


def kernel(**inputs):
    if "nc" not in _CACHE:
        _CACHE["nc"] = _build_device()
    nc = _CACHE["nc"]

    yT_np, wf_np, wh_r0, wh_r1, wh_r2 = _host_prep(**inputs)
    ones_np = np.ones((1, B), np.float16)
    in_maps = [{"yT": yT_np, "wflat": wf_np, "wh_r0": wh_r0[c],
                "wh_r1": wh_r1[c], "wh_r2": wh_r2[c], "ones": ones_np}
               for c in range(NCORES)]

    import os
    trace = bool(int(os.environ.get("KERNEL_TRACE", "0")))
    res = run_bass_kernel_spmd(nc, in_maps, core_ids=list(range(NCORES)),
                               trace=trace)
    _CACHE["last_result"] = res

    outs = [res.results[c]["out"].astype(np.float32) for c in range(NCORES)]
    glob = np.concatenate([o[:, :VG_SH] for o in outs], axis=1)
    sense = np.concatenate([o[:, VG_SH:] for o in outs], axis=1)
    return glob, sense
